# revision 1
# baseline (speedup 1.0000x reference)
"""VQ codebook decoder on 8 Trainium2 NeuronCores.

Strategy: data-parallel over tokens, tokens assigned to cores in globally
index-sorted order. Each core deduplicates its 2304 tokens to unique codebook
entries and runs the 2-layer MLP (1024 -> 4096 gelu -> 1024, bf16 on the
tensor engine, fp32 PSUM) only on unique rows:

  Ranks 0..NB*128-1 hold the core's non-single values (sorted, padded);
  the top S=48 ranks hold values referenced by exactly ONE token each
  ("singles"). Every core processes the same RU = NB*128 + S rows.

  Phase A: indirect-DMA gather unique codebook rows in 128-row slices
    (bf16, transposed layout), run the MLP; decoded blocks below NB*128 go
    to a DRAM dec table; the final S-row singles block maps 1:1 onto the
    last S tokens and is DMA'd straight to the output (no dec roundtrip,
    short tail). The singles block's mm2 runs with swapped operands
    (moving dim = its S rows, ~1/3 the cost of a full 128-row block) and
    is PE-transposed back via a bf16 identity.
  Phase B: per token block, indirect-DMA gather decoded rows by dedup rank
    (f32) and DMA to the output. Non-single ranks are assigned in
    descending frequency order, so most token blocks need only an early
    prefix of the dec table and every gather fires well before the end.

Head: mm1 h-tiles of the first 4 slices are emitted in one staggered
round-robin so the PE always has a runnable tile while W1 streams in
(w1 is host-tiled [p, ht, ks, 128] and loaded per h-tile, contiguous,
in mm1 consumption order); a few warmup matmuls ramp the PE clock during
the initial DMAs. The host applies the inverse token permutation when
unsharding.
"""

import sys

if "/opt/trn_rl_repo" not in sys.path:
    sys.path.insert(0, "/opt/trn_rl_repo")

import numpy as np
import ml_dtypes

import concourse.bass as bass
import concourse.mybir as mybir
import concourse.tile as tile
from concourse import bacc
from concourse.bass_utils import run_bass_kernel_spmd

B, M = 32, 576
CB, D, H, O = 8192, 1024, 4096, 1024
N_CORES = 8
T_TOTAL = B * M          # 18432
T = T_TOTAL // N_CORES   # 2304 tokens per core
P = 128
DK = D // P              # 8  k-subtiles for mm1
HK = H // P              # 32 k-subtiles for mm2
NO = O // 512            # 2  output column halves

U_PAD = 1024             # max unique codebook rows per core (dedup path)
S = 48                   # single-use ranks at the top (direct store)
TB = 128                 # token block size (Phase B granularity)
WARMUP_MM = 16           # dummy matmuls to ramp the PE clock at t=0
SLICE = 256              # phase-A slice granularity (rows per gather)
SWAPPED_TAIL = False     # swapped-operand mm2 for the singles block

BF16 = mybir.dt.bfloat16
F32 = mybir.dt.float32

_cache: dict = {}


def _wrap16(v):
    """int16 index layout for dma_gather: token j at [j%16, j//16], the
    16-row block replicated 8x down the 128 partitions."""
    v = np.asarray(v).astype(np.int16)
    return np.ascontiguousarray(np.tile(v.reshape(-1, 16).T, (8, 1)))


def _mlp_slice(nc, w1sb, w2sb, b1sb, b2sb, qpool, hpool, opool, p1pool,
               p2pool, cb16, idxsb, idx_col0, n_gather, n_real, store):
    """Gather n_gather codebook rows (idx cols at idx_col0), run the MLP on
    the first n_real of them; store(t2, chunk, osb) consumes each <=128-row
    fp32 output block."""
    qT = qpool.tile([P, DK, n_gather], BF16, name="qT")
    nc.gpsimd.dma_gather(
        qT[:, :, :], cb16[:, :],
        idxsb[:, idx_col0:idx_col0 + n_gather // 16],
        n_gather, n_gather, D, transpose=True,
    )
    hT = hpool.tile([P, HK, n_real], BF16, name="hT")
    for h in range(HK):
        ps1 = p1pool.tile([P, n_real], F32, name="ps1")
        for ks in range(DK):
            nc.tensor.matmul(
                ps1[:, :],
                w1sb[:, h, ks, 0:P],
                qT[:, ks, 0:n_real],
                start=(ks == 0), stop=(ks == DK - 1),
            )
        nc.scalar.activation(
            hT[:, h, :], ps1[:, :],
            mybir.ActivationFunctionType.Gelu_apprx_tanh,
            bias=b1sb[:, h:h + 1],
        )
    for t2 in range((n_real + P - 1) // P):
        chunk = min(P, n_real - t2 * P)
        osb = opool.tile([P, O], F32, name="osb")
        for o in range(NO):
            ps2 = p2pool.tile([P, 512], F32, name="ps2")
            for ks in range(HK):
                nc.tensor.matmul(
                    ps2[0:chunk, :],
                    hT[:, ks, t2 * P:t2 * P + chunk],
                    w2sb[:, ks, o * 512:(o + 1) * 512],
                    start=(ks == 0), stop=(ks == HK - 1),
                )
            nc.vector.tensor_add(
                osb[0:chunk, o * 512:(o + 1) * 512], ps2[0:chunk, :],
                b2sb[0:chunk, o * 512:(o + 1) * 512],
            )
        store(t2, chunk, osb)


def _declare_common(nc):
    cb16 = nc.declare_dram_parameter("cb16", [CB, D], BF16, isOutput=False)
    # w1 host-tiled [p, h-tile, ks, 128] so h-tile chunks are contiguous
    w1 = nc.declare_dram_parameter("w1", [P, HK, DK, P], BF16, isOutput=False)
    w2 = nc.declare_dram_parameter("w2", [H, O], BF16, isOutput=False)
    b1r = nc.declare_dram_parameter("b1r", [P, HK], F32, isOutput=False)
    b2r = nc.declare_dram_parameter("b2r", [P, O], BF16, isOutput=False)
    ident = nc.declare_dram_parameter("ident", [P, P], BF16, isOutput=False)
    return cb16, w1, w2, b1r, b2r, ident


def _load_weights(nc, wpool, w1, w2, b1r, b2r, rank16=None, ident=None):
    # w1 split per h-tile in mm1 consumption order; each chunk is contiguous
    # in both DRAM and SBUF (full bandwidth), and fine-grained so the PE
    # unblocks as early as possible while w1 streams in.
    # w1sb[p, ht, ks, c] = W1[ks*128+p, ht*128+c]
    w1sb = wpool.tile([P, HK, DK, P], BF16)
    nc.sync.dma_start(out=w1sb[:, 0:1], in_=w1[:, 0:1])
    nc.sync.dma_start(out=w1sb[:, 1:2], in_=w1[:, 1:2])
    b1sb = wpool.tile([P, HK], F32)
    nc.sync.dma_start(out=b1sb[:], in_=b1r[:])
    ranksb = None
    for ht in range(2, 8):
        nc.sync.dma_start(out=w1sb[:, ht:ht + 1], in_=w1[:, ht:ht + 1])
    if rank16 is not None:
        ranksb = wpool.tile([P, T // 16], mybir.dt.int16)
        nc.sync.dma_start(out=ranksb[:], in_=rank16[:])
    for ht in range(8, HK):
        nc.sync.dma_start(out=w1sb[:, ht:ht + 1], in_=w1[:, ht:ht + 1])
    b2sb = wpool.tile([P, O], BF16)
    nc.sync.dma_start(out=b2sb[:], in_=b2r[:])
    w2sb = wpool.tile([P, HK, O], BF16)
    w2v = w2.rearrange("(ks p) o -> p ks o", p=P)
    for ks in range(0, HK, 4):
        nc.sync.dma_start(out=w2sb[:, ks:ks + 4, :], in_=w2v[:, ks:ks + 4, :])
    identsb = None
    if ident is not None:
        identsb = wpool.tile([P, P], BF16)
        nc.sync.dma_start(out=identsb[:], in_=ident[:])
    return w1sb, w2sb, b1sb, b2sb, ranksb, identsb


def _slice_plan(ru):
    """Phase-A slices as (row0, n_gather, n_real). SLICE-row granularity:
    bigger slices mean fewer, longer matmuls (the PE sequencer costs ~a
    fixed overhead per instruction on HW); gathers pad to 128 multiples."""
    n_cover = ((ru + P - 1) // P) * P
    plan = []
    for row in range(0, n_cover, SLICE):
        if ru - row > 0:
            plan.append((row, min(SLICE, n_cover - row),
                         min(SLICE, ru - row)))
    return plan


def _tok_blocks(n_gathered):
    sizes = [TB] * (n_gathered // TB)
    if n_gathered % TB:
        sizes.append(n_gathered % TB)
    return sizes


def _build_dedup(schedule, nb, repeats: int = 1):
    """schedule[i] = number of 128-row dec blocks gathered token block i
    needs; nb = number of gathered dec blocks (ranks >= nb*128 are singles
    stored straight to the output)."""
    ru = nb * P + S
    slices = _slice_plan(ru)
    tsizes = _tok_blocks(T - S)
    assert len(schedule) == len(tsizes)
    nc = bacc.Bacc("TRN2", target_bir_lowering=False, debug=False,
                   num_devices=N_CORES)
    cb16, w1, w2, b1r, b2r, ident = _declare_common(nc)
    uidx16 = nc.declare_dram_parameter("uidx16", [P, U_PAD // 16],
                                       mybir.dt.int16, isOutput=False)
    rank16 = nc.declare_dram_parameter("rank16", [P, T // 16],
                                       mybir.dt.int16, isOutput=False)
    out = nc.declare_dram_parameter("out", [T, O], F32, isOutput=True)

    with tile.TileContext(nc) as tc:
        with (
            tc.tile_pool(name="wpool", bufs=1) as wpool,
            tc.tile_pool(name="qpool", bufs=3) as qpool,
            tc.tile_pool(name="hpool", bufs=3) as hpool,
            tc.tile_pool(name="opool", bufs=2) as opool,
            tc.tile_pool(name="g2pool", bufs=2) as g2pool,
            tc.tile_pool(name="dpool", bufs=1, space="DRAM") as dpool,
            tc.tile_pool(name="tpool", bufs=1) as tpool,
            tc.tile_pool(name="p1pool", bufs=4, space="PSUM") as p1pool,
            tc.tile_pool(name="p2pool", bufs=2, space="PSUM") as p2pool,
            tc.tile_pool(name="p4pool", bufs=2, space="PSUM") as p4pool,
        ):
          for _rep in range(repeats):
            uidxsb = wpool.tile([P, U_PAD // 16], mybir.dt.int16)
            nc.sync.dma_start(out=uidxsb[:], in_=uidx16[:])

            qts, hts = {}, {}

            def gather(si):
                row0, n_g, _ = slices[si]
                qT = qpool.tile([P, DK, n_g], BF16, name="qT")
                nc.gpsimd.dma_gather(
                    qT[:, :, :], cb16[:, :],
                    uidxsb[:, row0 // 16:(row0 + n_g) // 16],
                    n_g, n_g, D, transpose=True,
                )
                qts[si] = qT

            gather(0)
            if WARMUP_MM:
                # ramp the PE clock while the first gather + W1 chunk land
                ub = uidxsb.bitcast(BF16)
                wm = p2pool.tile([P, 64], F32, name="ps2")
                for _ in range(WARMUP_MM):
                    nc.tensor.matmul(wm[0:64, 0:64], ub[:, 0:64],
                                     ub[:, 0:64], start=True, stop=True)
            w1sb, w2sb, b1sb, b2sb, ranksb, identsb = _load_weights(
                nc, wpool, w1, w2, b1r, b2r, rank16, ident)
            dec = dpool.tile([nb * P, O], F32)

            tok0 = [sum(tsizes[:i]) for i in range(len(tsizes))]

            def emit_tok_block(i):
                # schedule[i] counts 128-row dec blocks this token block needs
                need = P * schedule[i]
                nt = tsizes[i]
                g2 = g2pool.tile([P, 1, O], F32, name="g2")
                nc.gpsimd.dma_gather(
                    g2[:, :, :], dec[0:need, :],
                    ranksb[:, tok0[i] // 16:(tok0[i] + nt) // 16],
                    nt, nt, O,
                )
                nc.sync.dma_start(out=out[tok0[i]:tok0[i] + nt, :],
                                  in_=g2[0:nt, 0, :])

            state = {"emitted": 0, "dec_done": 0}

            def after_store():
                state["dec_done"] += 1
                while (state["emitted"] < len(tsizes)
                       and schedule[state["emitted"]] <= state["dec_done"]):
                    emit_tok_block(state["emitted"])
                    state["emitted"] += 1

            def mm1_tile(si, h):
                _, _, n_r = slices[si]
                ps1 = p1pool.tile([P, n_r], F32, name="ps1")
                for ks in range(DK):
                    nc.tensor.matmul(
                        ps1[:, :],
                        w1sb[:, h, ks, 0:P],
                        qts[si][:, ks, 0:n_r],
                        start=(ks == 0), stop=(ks == DK - 1),
                    )
                nc.scalar.activation(
                    hts[si][:, h, :], ps1[:, :],
                    mybir.ActivationFunctionType.Gelu_apprx_tanh,
                    bias=b1sb[:, h:h + 1],
                )

            def mm2_swapped_tail(si, c0):
                # Singles block (S rows, ranks map 1:1 onto the last S
                # tokens): swapped operands make the moving dim the S rows
                # (cost ~S instead of a full 128-row block), output comes
                # out o-major and is PE-transposed back.
                osbT = tpool.tile([P, O // P, S], BF16, name="osbT")
                for ot in range(O // P):
                    psT = p1pool.tile([P, S], F32, name="ps1")
                    for ks in range(HK):
                        nc.tensor.matmul(
                            psT[:, :],
                            w2sb[:, ks, ot * P:(ot + 1) * P],
                            hts[si][:, ks, c0:c0 + S],
                            start=(ks == 0), stop=(ks == HK - 1),
                        )
                    nc.scalar.activation(osbT[:, ot, :], psT[:, :],
                                         mybir.ActivationFunctionType.Copy)
                osb = opool.tile([P, O], F32, name="osb")
                for op in range(O // P // 2):
                    pst2 = p4pool.tile([P, 2 * P], BF16, name="pst2")
                    for j in range(2):
                        nc.tensor.transpose(pst2[0:S, j * P:(j + 1) * P],
                                            osbT[:, 2 * op + j, :],
                                            identsb[:, :])
                    nc.vector.tensor_add(
                        osb[0:S, 2 * op * P:(2 * op + 2) * P],
                        pst2[0:S, :],
                        b2sb[0:S, 2 * op * P:(2 * op + 2) * P])
                nc.sync.dma_start(out=out[T - S:T, 0:512],
                                  in_=osb[0:S, 0:512])
                nc.scalar.dma_start(out=out[T - S:T, 512:O],
                                    in_=osb[0:S, 512:O])

            def mm2_slice(si):
                row0, _, n_r = slices[si]
                for t2 in range((n_r + P - 1) // P):
                    row = row0 + t2 * P
                    if row >= nb * P and SWAPPED_TAIL:
                        mm2_swapped_tail(si, t2 * P)
                        continue
                    chunk = min(P, n_r - t2 * P)
                    singles = row >= nb * P
                    osb = opool.tile([P, O], F32, name="osb")
                    for o in range(NO):
                        ps2 = p2pool.tile([P, 512], F32, name="ps2")
                        for ks in range(HK):
                            nc.tensor.matmul(
                                ps2[0:chunk, :],
                                hts[si][:, ks, t2 * P:t2 * P + chunk],
                                w2sb[:, ks, o * 512:(o + 1) * 512],
                                start=(ks == 0), stop=(ks == HK - 1),
                            )
                        if singles:
                            # singles: ranks map 1:1 onto the last S tokens;
                            # quarter-grain adds + outs on alternating hwdge
                            # rings so the final hop off-chip is short
                            for q in range(2):
                                c0 = o * 512 + q * 256
                                nc.vector.tensor_add(
                                    osb[0:chunk, c0:c0 + 256],
                                    ps2[0:chunk, q * 256:(q + 1) * 256],
                                    b2sb[0:chunk, c0:c0 + 256],
                                )
                                eng = nc.sync if q == 0 else nc.scalar
                                eng.dma_start(
                                    out=out[T - S:T, c0:c0 + 256],
                                    in_=osb[0:chunk, c0:c0 + 256])
                        else:
                            nc.vector.tensor_add(
                                osb[0:chunk, o * 512:(o + 1) * 512],
                                ps2[0:chunk, :],
                                b2sb[0:chunk, o * 512:(o + 1) * 512],
                            )
                    if not singles:
                        nc.sync.dma_start(out=dec[row:row + chunk, :],
                                          in_=osb[0:chunk, :])
                        after_store()

            # Head slices run their mm1 h-tiles interleaved in one static
            # round-robin so the PE can always advance with whichever w1
            # h-tile chunk has landed (a single slice underruns the w1
            # stream). Later slices are staggered a couple of tiles so their
            # gathers have time to land.
            nhead = min(3, len(slices))
            for si in range(1, nhead):
                gather(si)
            for si in range(nhead):
                _, _, n_r = slices[si]
                hts[si] = hpool.tile([P, HK, n_r], BF16, name="hT")
            offset = [0, 2, 4, 6]
            for hh in range(HK + offset[nhead - 1]):
                for si in range(nhead):
                    h = hh - offset[si]
                    if 0 <= h < HK:
                        mm1_tile(si, h)
            for si in range(nhead, len(slices)):
                gather(si)
            # Drain: alternate mm2 of finished slices (frees hT slots) with
            # mm1 of remaining slices.
            done = list(range(nhead))
            for si in range(nhead, len(slices)):
                mm2_slice(done.pop(0))
                _, _, n_r = slices[si]
                hts[si] = hpool.tile([P, HK, n_r], BF16, name="hT")
                for h in range(HK):
                    mm1_tile(si, h)
                done.append(si)
            for si in done:
                mm2_slice(si)
            while state["emitted"] < len(tsizes):
                emit_tok_block(state["emitted"])
                state["emitted"] += 1

    nc.compile()
    return nc


def _build_dense(repeats: int = 1):
    """Fallback: straight data-parallel, no dedup (2304 tokens per core)."""
    nc = bacc.Bacc("TRN2", target_bir_lowering=False, debug=False,
                   num_devices=N_CORES)
    cb16, w1, w2, b1r, b2r, _ = _declare_common(nc)
    idx16 = nc.declare_dram_parameter("idx16", [P, T // 16], mybir.dt.int16,
                                      isOutput=False)
    out = nc.declare_dram_parameter("out", [T, O], F32, isOutput=True)
    TS = 256

    with tile.TileContext(nc) as tc:
        with (
            tc.tile_pool(name="wpool", bufs=1) as wpool,
            tc.tile_pool(name="qpool", bufs=2) as qpool,
            tc.tile_pool(name="hpool", bufs=2) as hpool,
            tc.tile_pool(name="opool", bufs=3) as opool,
            tc.tile_pool(name="p1pool", bufs=4, space="PSUM") as p1pool,
            tc.tile_pool(name="p2pool", bufs=2, space="PSUM") as p2pool,
        ):
          for _rep in range(repeats):
            idxsb = wpool.tile([P, T // 16], mybir.dt.int16)
            nc.sync.dma_start(out=idxsb[:], in_=idx16[:])
            w1sb, w2sb, b1sb, b2sb, _, _ = _load_weights(nc, wpool, w1, w2,
                                                         b1r, b2r)
            for i in range(T // TS):
                def store(t2, chunk, osb, i=i):
                    row = i * TS + t2 * P
                    nc.sync.dma_start(out=out[row:row + chunk, :],
                                      in_=osb[0:chunk, :])
                _mlp_slice(nc, w1sb, w2sb, b1sb, b2sb, qpool, hpool, opool,
                           p1pool, p2pool, cb16, idxsb, i * (TS // 16),
                           TS, TS, store)

    nc.compile()
    return nc


def _get_nc(kind, schedule=None, nb=None, repeats=1):
    key = (kind, schedule, nb, repeats)
    if key not in _cache:
        if kind == "dedup":
            _cache[key] = _build_dedup(schedule, nb, repeats)
        else:
            _cache[key] = _build_dense(repeats)
    return _cache[key]


def _prep_weights(codebook, W1, b1, W2, b2):
    bf = ml_dtypes.bfloat16
    return {
        "cb16": np.ascontiguousarray(codebook.astype(bf)),
        "w1": np.ascontiguousarray(
            W1.astype(bf).reshape(DK, P, HK, P).transpose(1, 2, 0, 3)),
        "w2": np.ascontiguousarray(W2.astype(bf)),
        "b1r": np.ascontiguousarray(b1.astype(np.float32).reshape(HK, P).T),
        "b2r": np.ascontiguousarray(
            np.broadcast_to(b2.astype(bf)[None, :], (P, O))),
        "ident": np.eye(P, dtype=bf),
    }


def _plan_dedup(index):
    """Sorted-index sharding + per-core dedup with single-use values in the
    top S ranks. Returns None if infeasible (caller uses the dense kernel)."""
    idx_flat = np.asarray(index).reshape(-1)
    order = np.argsort(idx_flat, kind="stable")
    cores = []
    u_eff_max = 0
    for c in range(N_CORES):
        perm = order[c * T:(c + 1) * T]
        vals = idx_flat[perm]
        uniq, counts = np.unique(vals, return_counts=True)
        if uniq.size > U_PAD or (counts == 1).sum() < S:
            return None
        singles = uniq[counts == 1][-S:]          # S largest single values
        keep = ~np.isin(uniq, singles, assume_unique=True)
        rest, rcounts = uniq[keep], counts[keep]
        # Descending-frequency rank order: frequent values get low ranks, so
        # most token blocks only need an early prefix of the dec table and
        # the final dec block is referenced by as few tokens as possible.
        order_r = np.argsort(-rcounts, kind="stable")
        rest = rest[order_r]
        u_eff_max = max(u_eff_max, rest.size)
        cores.append((perm, vals, rest, singles))
    nb = (u_eff_max + P - 1) // P
    if nb * P + S > U_PAD:
        return None
    tsizes = _tok_blocks(T - S)
    perms, uidxs, ranks, needs = [], [], [], []
    for perm, vals, rest, singles in cores:
        up = np.zeros(U_PAD, np.int64)
        up[:rest.size] = rest
        up[nb * P:nb * P + S] = singles
        rank_of = {v: i for i, v in enumerate(rest)}
        rank_of.update({v: nb * P + i for i, v in enumerate(singles)})
        tok_rank = np.array([rank_of[v] for v in vals], np.int64)
        # gathered (rank < nb*P) tokens first in rank order, singles last
        tperm = np.argsort(np.where(tok_rank >= nb * P,
                                    tok_rank + T_TOTAL, tok_rank),
                           kind="stable")
        perms.append(perm[tperm])
        inv = tok_rank[tperm]
        uidxs.append(_wrap16(up))
        ranks.append(_wrap16(inv))
        need, t0 = [], 0
        for nt in tsizes:
            need.append(int(np.ceil((inv[t0:t0 + nt].max() + 1) / P)))
            t0 += nt
        needs.append(need)
    schedule = tuple(max(needs[c][i] for c in range(N_CORES))
                     for i in range(len(tsizes)))
    assert max(schedule) <= nb
    return perms, uidxs, ranks, schedule, nb


def kernel(index, codebook, W1, b1, W2, b2):
    wmaps = _prep_weights(codebook, W1, b1, W2, b2)
    plan = _plan_dedup(index)
    if plan is not None:
        perms, uidxs, ranks, schedule, nb = plan
        nc = _get_nc("dedup", schedule, nb)
        in_maps = [{**wmaps, "uidx16": uidxs[c], "rank16": ranks[c]}
                   for c in range(N_CORES)]
        res = run_bass_kernel_spmd(nc, in_maps, list(range(N_CORES)))
        out = np.empty((T_TOTAL, O), np.float32)
        for c in range(N_CORES):
            out[perms[c]] = res.results[c]["out"]
    else:
        nc = _get_nc("dense")
        idx_flat = np.asarray(index).reshape(-1)
        in_maps = [{**wmaps, "idx16": _wrap16(idx_flat[c * T:(c + 1) * T])}
                   for c in range(N_CORES)]
        res = run_bass_kernel_spmd(nc, in_maps, list(range(N_CORES)))
        out = np.concatenate([res.results[c]["out"] for c in range(N_CORES)],
                             axis=0)
    return out.reshape(B, M, O).astype(np.float32)

